# revision 63
# baseline (speedup 1.0000x reference)
"""Trainium2 Bass kernel for PVT-style spatial-reduction attention.

Problem (per batch element b, data-parallel over B=8 on 8 NeuronCores):
  q   = x @ Wq                               [N=16384, 64]
  xsr = conv(x as [64,128,128], k=s=8) + b   [256, 64]
  z   = layernorm(xsr) (affine folded)       [256, 64]
  k   = z @ Wk ;  v = z @ Wv
  out = softmax(0.125 * q k^T) v @ Wproj + bproj

Algebraic folds (host side, exact):
  scores = q k^T * 0.125 = x @ k2^T where k2 = z @ Wkq + bkq,
      Wkq = 0.125 * diag(g) Wk' Wq^T folded      (kills q projection)
  probs @ (v @ Wproj + 1 bproj^T) = out          (kills out projection;
      softmax rows sum to 1 so bproj rides along in v'')
  LN affine (g, b) folded into Wkv; LN on device is standardize-only.

Device layout: attention runs transposed (scores^T = k2 @ x^T) so softmax'
exp feeds the PV matmul as the stationary operand with no transposes in the
inner loop; the softmax denominator comes from a ones-column in v''.
"""

import os
import sys

import numpy as np
import ml_dtypes

for _p in ("/opt/trn_rl_repo", "/root/.axon_site/_ro/trn_rl_repo"):
    if os.path.isdir(_p) and _p not in sys.path:
        sys.path.insert(0, _p)

B = 8
N = 16384          # 128*128 image
C = 64
NK = 256           # 16*16 patches
SR = 8
NT = N // 128      # 128 query tiles of 128 rows
QG = 1024          # queries per main-loop group
NG = N // QG       # 16 groups
SCALE = C ** -0.5  # 0.125

LAST_RESULT = None  # test harness reads exec_time_ns from here

_CACHED_NC = None


def _build_nc():
    import concourse.bass as bass
    import concourse.tile as tile
    from concourse import bacc, mybir

    f32 = mybir.dt.float32
    bf16 = mybir.dt.bfloat16
    AF = mybir.ActivationFunctionType
    ALU = mybir.AluOpType
    PSUM = bass.MemorySpace.PSUM

    nc = bacc.Bacc("TRN2", target_bir_lowering=False, debug=False)

    x_d = nc.dram_tensor("x", [N, C], f32, kind="ExternalInput")
    wc2_d = nc.dram_tensor("wc2", [128, 32, 64], bf16, kind="ExternalInput")
    wkq_d = nc.dram_tensor("wkq", [64, 64], bf16, kind="ExternalInput")
    bkq_d = nc.dram_tensor("bkq", [64, 1], f32, kind="ExternalInput")
    wvp_d = nc.dram_tensor("wvp", [64, 64], bf16, kind="ExternalInput")
    bvp_d = nc.dram_tensor("bvp", [1, 64], f32, kind="ExternalInput")
    srb_d = nc.dram_tensor("srb", [64, 1], f32, kind="ExternalInput")
    idbf_d = nc.dram_tensor("idbf", [128, 128], bf16, kind="ExternalInput")
    idf_d = nc.dram_tensor("idf", [64, 64], f32, kind="ExternalInput")
    out_d = nc.dram_tensor("out", [N, C], f32, kind="ExternalOutput")

    with tile.TileContext(nc) as tc:
        with tc.tile_pool(name="const", bufs=1) as constp:
            wc2 = constp.tile([128, 32, 64], bf16)
            wkq = constp.tile([64, 64], bf16)
            bkq = constp.tile([64, 1], f32)
            wvp = constp.tile([64, 64], bf16)
            srb = constp.tile([64, 1], f32)
            bvp = constp.tile([128, 64], f32)
            id_bf = constp.tile([128, 128], bf16)
            id_f32 = constp.tile([64, 64], f32)

            # long-lived small tensors for the attention loop
            # k2^T duplicated into both partition halves (PE needs lhsT and
            # rhs at the same base partition; odd-tile queries sit at 64:128)
            k2T = constp.tile([128, 256], bf16)
            v_aug0 = constp.tile([128, 65], bf16)    # [v'' | 1] keys 0:128
            v_aug1 = constp.tile([128, 65], bf16)    # [v'' | 1] keys 128:256
            # x^T, bf16: partitions 0:64 = channels of even tiles,
            # 64:128 = channels of odd tiles; free = (tile//2)*128 + row
            xT = constp.tile([128, N // 2], bf16)

            # ---- phase 1: stream x, cast to bf16, PE-transpose into xT.
            # The conv halves are emitted mid-stream so the PE reaches them
            # while the later x groups are still loading (the PE executes
            # its queue in order). xT free f = 512*i' + 128*m + 8*j' + dw.
            xt_conv = xT[:, :].rearrange(
                "p (i b j w) -> p i b j w", i=16, b=4, j=16, w=8)
            with (
                tc.tile_pool(name="stage", bufs=3) as stage,
                tc.tile_pool(name="stageps", bufs=4, space=PSUM) as stageps,
                tc.tile_pool(name="convps", bufs=1, space=PSUM) as convps,
            ):
                xsr = constp.tile([64, 256], f32)

                def conv_quarter(iq):
                    isl = slice(iq * 4, iq * 4 + 4)
                    xsrT_ps = convps.tile([64, 4, 16], f32, name=f"xsr{iq}")
                    for m in range(4):
                        for dw in range(8):
                            idx = m * 8 + dw
                            nc.tensor.matmul(
                                xsrT_ps[:],
                                wc2[:, idx, :],
                                xt_conv[:, isl, m, :, dw],
                                start=(idx == 0),
                                stop=(idx == 31),
                            )
                    # + conv bias (DVE so the ACT engine only ever runs
                    # Ln/Exp -> 2 table loads total)
                    nc.vector.tensor_scalar_add(
                        xsr[:, iq * 64:(iq + 1) * 64],
                        xsrT_ps[:].rearrange("p a b -> p (a b)"), srb[:])

                for g in range(NG):
                    xf = stage.tile([128, 8, 64], f32, bufs=6)
                    eng = nc.sync if g % 2 == 0 else nc.scalar
                    eng.dma_start(
                        xf[:],
                        x_d[g * QG:(g + 1) * QG, :].rearrange(
                            "(t p) c -> p t c", p=128),
                    )
                    if g == 0:
                        # constants, behind the first x-load issue
                        nc.sync.dma_start(id_bf[:], idbf_d[:])
                        nc.scalar.dma_start(wc2[:], wc2_d[:])
                        nc.sync.dma_start(id_f32[:], idf_d[:])
                        nc.scalar.dma_start(wkq[:], wkq_d[:])
                        nc.sync.dma_start(bkq[:], bkq_d[:])
                        nc.scalar.dma_start(wvp[:], wvp_d[:])
                        nc.sync.dma_start(srb[:], srb_d[:])
                        nc.scalar.dma_start(bvp[:],
                                            bvp_d[:].to_broadcast((128, 64)))
                    xb = stage.tile([128, 8, 64], bf16)
                    nc.vector.tensor_copy(xb[:], xf[:])
                    xt_ps = stageps.tile([128, 512], bf16)
                    for u in range(4):  # tile pairs (2 tiles per transpose)
                        nc.tensor.transpose(xt_ps[:, u * 128:(u + 1) * 128],
                                            xb[:, 2 * u:2 * u + 2, :],
                                            id_bf[:])
                    if g % 2 == 0:
                        nc.scalar.copy(xT[:, g * 512:(g + 1) * 512],
                                       xt_ps[:])
                    else:
                        nc.vector.tensor_copy(xT[:, g * 512:(g + 1) * 512],
                                              xt_ps[:])
                    # conv quarter iq needs only x groups < 4*(iq+1)
                    if g in (3, 7, 11, 15):
                        conv_quarter(g // 4)

            # ---- phase 2: LN + k/v (small)
            with (
                tc.tile_pool(name="p2sb", bufs=1) as p2sb,
                tc.tile_pool(name="p2ps", bufs=1, space=PSUM) as p2ps,
            ):
                # ---- phase 2b: LN (standardize; affine folded into weights)
                eps = p2sb.tile([128, 1], f32)
                nc.vector.memset(eps[:], 1e-5)
                zn_ps, mv = [], []
                for h in range(2):
                    zp = p2ps.tile([128, 64], f32, bufs=2)
                    nc.tensor.transpose(zp[:], xsr[:, h * 128:(h + 1) * 128],
                                        id_f32[:64, :64])
                    stats = p2sb.tile([128, 6], f32)
                    nc.vector.bn_stats(stats[:], zp[:])
                    m = p2sb.tile([128, 2], f32)
                    nc.vector.bn_aggr(m[:], stats[:])
                    zn_ps.append(zp)
                    mv.append(m)
                # rstd = exp(-0.5 * ln(var + eps)) — single Ln and single
                # Exp instruction so the ACT table set switches exactly
                # twice (natural_log -> exp_and_others)
                var2 = p2sb.tile([128, 2], f32)
                for h in range(2):
                    nc.vector.tensor_copy(var2[:, h:h + 1], mv[h][:, 1:2])
                lnv = p2sb.tile([128, 2], f32)
                nc.scalar.activation(lnv[:], var2[:], AF.Ln, bias=eps[:])
                rstd = p2sb.tile([128, 2], f32)
                nc.scalar.activation(rstd[:], lnv[:], AF.Exp, scale=-0.5)
                zsb = []
                for h in range(2):
                    negmu = p2sb.tile([128, 1], f32)
                    nc.vector.tensor_scalar_mul(negmu[:], mv[h][:, 0:1], -1.0)
                    z = p2sb.tile([128, 64], bf16)
                    nc.vector.tensor_scalar(z[:], zn_ps[h][:], negmu[:],
                                            rstd[:, h:h + 1], ALU.add,
                                            ALU.mult)
                    zsb.append(z)

                zT = p2sb.tile([64, 256], bf16)
                for h in range(2):
                    zT_ps = p2ps.tile([64, 128], bf16)
                    nc.tensor.transpose(zT_ps[:], zsb[h][:], id_bf[:])
                    nc.vector.tensor_copy(zT[:, h * 128:(h + 1) * 128], zT_ps[:])

                # ---- phase 2c: k2^T = Wkq^T z^T + bkq ; v'' = z Wvp + bvp
                k2_ps = p2ps.tile([64, 256], f32)
                nc.tensor.matmul(k2_ps[:], wkq[:], zT[:])
                nc.vector.tensor_scalar_add(k2T[0:64, :], k2_ps[:], bkq[:])
                nc.sync.dma_start(k2T[64:128, :], k2T[0:64, :])
                for kh, vt in ((0, v_aug0), (1, v_aug1)):
                    v2_ps = p2ps.tile([128, 64], f32)
                    nc.tensor.matmul(v2_ps[:], zT[:, kh * 128:(kh + 1) * 128],
                                     wvp[:])
                    nc.vector.tensor_tensor(vt[:, 0:64], v2_ps[:], bvp[:],
                                            ALU.add)
                    nc.vector.memset(vt[:, 64:65], 1.0)

            # ---- phase 3: attention main loop
            with (
                tc.tile_pool(name="msb", bufs=4) as msb,
                tc.tile_pool(name="mps_st", bufs=3, space=PSUM) as mps_st,
                tc.tile_pool(name="mps_pv", bufs=2, space=PSUM) as mps_pv,
            ):
                def scores(g):
                    eT = []
                    for kh in range(2):
                        st = mps_st.tile([128, QG], f32)  # 2 psum banks
                        for par in range(2):  # even/odd query tiles
                            nc.tensor.matmul(
                                st[:, par * 512:(par + 1) * 512],
                                k2T[64 * par:64 * par + 64,
                                    kh * 128:(kh + 1) * 128],
                                xT[64 * par:64 * par + 64,
                                   g * 512:(g + 1) * 512],
                            )
                        e = msb.tile([128, QG], bf16, bufs=6)
                        nc.scalar.activation(e[:], st[:], AF.Exp)
                        eT.append(e)
                    return eT

                # software pipeline: emit group g+1's score matmuls before
                # group g's PV so the in-order PE queue never stalls on exp
                eT_next = scores(0)
                for g in range(NG):
                    eT = eT_next
                    if g + 1 < NG:
                        eT_next = scores(g + 1)
                    for half in range(2):
                        pv = mps_pv.tile([128, 4, 65], f32)  # 1 psum bank
                        for cc in range(4):
                            ci = half * 4 + cc
                            for kh, vt in ((0, v_aug0), (1, v_aug1)):
                                nc.tensor.matmul(
                                    pv[:, cc, :],
                                    eT[kh][:, ci * 128:(ci + 1) * 128],
                                    vt[:],
                                    start=(kh == 0),
                                    stop=(kh == 1),
                                )
                        rr = msb.tile([128, 4, 1], f32)
                        nc.vector.reciprocal(rr[:], pv[:, :, 64:65])
                        outs = msb.tile([128, 4, 64], f32)
                        nc.vector.tensor_tensor(
                            outs[:], pv[:, :, 0:64],
                            rr[:].to_broadcast((128, 4, 64)), ALU.mult)
                        # one DMA stores the half's 4 tiles (t = 8g+half+2cc)
                        dview = out_d[g * QG:(g + 1) * QG, :].rearrange(
                            "(cc hf p) c -> p cc hf c", cc=4, hf=2,
                            p=128)[:, :, half, :]
                        eng = nc.sync if half == 0 else nc.scalar
                        eng.dma_start(dview, outs[:])

    nc.compile()
    return nc


def _host_fold(Wq, Wkv, Wproj, bproj, sr_w, sr_b, ln_g, ln_b):
    """Fold LN affine / q-proj / out-proj into small weight matrices."""
    f = np.float32
    Wq = np.asarray(Wq, f)
    Wkv = np.asarray(Wkv, f)
    Wproj = np.asarray(Wproj, f)
    bproj = np.asarray(bproj, f)
    sr_w = np.asarray(sr_w, f)
    sr_b = np.asarray(sr_b, f)
    g = np.asarray(ln_g, f)
    b = np.asarray(ln_b, f)

    Wkv_g = Wkv * g[:, None]
    bkv = b @ Wkv
    Wk, bk = Wkv_g[:, :C], bkv[:C]
    Wv, bv = Wkv_g[:, C:], bkv[C:]

    Wkq = SCALE * (Wk @ Wq.T)          # [in_c, key_c]
    bkq = SCALE * (bk @ Wq.T)          # [key_c]
    Wvp = Wv @ Wproj                   # [in_c, out_c]
    bvp = bv @ Wproj + bproj           # [out_c]

    wc2 = np.zeros((128, 32, 64), f)   # [(parity, c), m*8+dw, out_c]
    for m in range(4):
        for dw in range(8):
            idx = m * 8 + dw
            wc2[:64, idx, :] = sr_w[:, :, 2 * m, dw].T
            wc2[64:, idx, :] = sr_w[:, :, 2 * m + 1, dw].T

    bf = ml_dtypes.bfloat16
    return {
        "wc2": wc2.astype(bf),
        "wkq": Wkq.astype(bf),
        "bkq": bkq.reshape(64, 1).astype(f),
        "wvp": Wvp.astype(bf),
        "bvp": bvp.reshape(1, 64).astype(f),
        "srb": sr_b.reshape(64, 1).astype(f),
        "idbf": np.eye(128, dtype=bf),
        "idf": np.eye(64, dtype=f),
    }


def kernel(x, Wq, Wkv, Wproj, bproj, sr_w, sr_b, ln_g, ln_b, H=128, W=128):
    global _CACHED_NC, LAST_RESULT
    from concourse.bass_utils import run_bass_kernel_spmd

    x = np.asarray(x, np.float32)
    weights = _host_fold(Wq, Wkv, Wproj, bproj, sr_w, sr_b, ln_g, ln_b)

    if _CACHED_NC is None:
        _CACHED_NC = _build_nc()
    nc = _CACHED_NC

    in_maps = [{"x": np.ascontiguousarray(x[b]), **weights} for b in range(B)]
    res = run_bass_kernel_spmd(nc, in_maps, core_ids=list(range(B)))
    LAST_RESULT = res
    return np.stack([res.results[c]["out"] for c in range(B)]).astype(np.float32)
